# revision 33
# baseline (speedup 1.0000x reference)
"""Trainium2 Bass kernel for ArcDecoder pair scoring.

Reference computation (N=768 nodes, H=128 features):
    pairs (i, j), i != j:  out[i,j] = W2 @ relu(W1a @ z_i + W1b @ z_j + b1) + b2
where W1 = [W1a | W1b] ([128, 256] split along input dim).

Strategy (pure data parallel over 8 NeuronCores, no collectives):
  - core c owns i-rows [96c, 96c+96); output rows are contiguous in the
    final pair-major ordering, so gather = concat + drop diagonal.
  - device layout puts the hidden dim k on SBUF partitions:
      Abias[k, a] = W1a @ z_loc.T + b1     (per-core [128, 96])
      Bt[k, j]    = W1b @ z.T              (replicated [128, 768], bf16)
    per a: H_a = relu(Bt + Abias[:, a])    (split 66:30 DVE / ACT)
    out[a, :] = W2 @ H_a (+ b2 via a K=1 matmul into the same PSUM
    accumulation).  TensorE uses a zero-padded W2 stationary and 3
    column groups running concurrently; the row for a = 3r + g lands on
    PSUM partition 32g + (r % 16).  Two PSUM generations (16 rounds
    each) so eviction/output-DMA of gen 0 overlaps gen 1 compute.
"""

import numpy as np
import ml_dtypes

import concourse.bass as bass
import concourse.tile as tile
from concourse import bacc, mybir
from concourse.bass_utils import run_bass_kernel_spmd

N = 768
H = 128
NCORES = 8
ROWS = N // NCORES          # 96 i-rows per core
NGRP = 3                    # PE column groups (PSUM partitions 32g..32g+31)
RND = ROWS // NGRP          # 32 rounds; round r, group g handles a = 3r + g
GENS = 2
GRND = RND // GENS          # 16 rounds per PSUM generation
HALF = N // 2               # 384, PSUM bank limit for f32 is 512

_F32 = mybir.dt.float32
_BF16 = mybir.dt.bfloat16

_cache = {}


def _build():
    nc = bacc.Bacc(
        "TRN2",
        target_bir_lowering=False,
        debug=False,
        enable_asserts=False,
        num_devices=NCORES,
    )

    zT_d = nc.dram_tensor("zT", [H, N], _BF16, kind="ExternalInput")
    zTloc_d = nc.dram_tensor("zTloc", [H, ROWS], _BF16, kind="ExternalInput")
    w1aT_d = nc.dram_tensor("w1aT", [H, H], _BF16, kind="ExternalInput")
    w1bT_d = nc.dram_tensor("w1bT", [H, H], _BF16, kind="ExternalInput")
    b1col_d = nc.dram_tensor("b1col", [H, 1], _F32, kind="ExternalInput")
    b2row_d = nc.dram_tensor("b2row", [1, ROWS], _BF16, kind="ExternalInput")
    S_d = nc.dram_tensor("S", [H, GRND, 32], _BF16, kind="ExternalInput")
    out_d = nc.dram_tensor("out", [ROWS, N], _F32, kind="ExternalOutput")

    relu = mybir.ActivationFunctionType.Relu
    copyf = mybir.ActivationFunctionType.Copy
    add_op = mybir.AluOpType.add
    max_op = mybir.AluOpType.max

    with tile.TileContext(nc) as tc:
        with (
            tc.tile_pool(name="const", bufs=1) as cpool,
            tc.tile_pool(name="hpool", bufs=8) as hpool,
            tc.tile_pool(name="opool", bufs=2) as opool,
            tc.tile_pool(name="psA", bufs=1, space=bass.MemorySpace.PSUM) as psA,
            tc.tile_pool(name="psum", bufs=2, space=bass.MemorySpace.PSUM) as pspool,
        ):
            # ACT spline-table prewarm (Relu + Copy) so any one-time
            # ACT_TABLE_LOAD overlaps the input DMAs.
            scratch = cpool.tile([1, 8], _F32, tag="scratch")
            nc.gpsimd.memset(scratch[:], 0.0)
            nc.scalar.activation(scratch[:], scratch[:], relu)
            nc.scalar.activation(scratch[:], scratch[:], copyf)

            # all-ones moving row for the b2 fold-in matmul
            ones_sb = cpool.tile([1, HALF], _BF16)
            nc.gpsimd.memset(ones_sb[:], 1.0)

            # inputs, spread across the two HWDGE queues (sync, scalar).
            # Tile's per-queue clock makes a consumer wait for every DMA
            # emitted before it in program order, so emit the setup matmuls
            # immediately after the DMAs they actually need and push the
            # small late-use DMAs (b1col/b2row/S) after them.
            zTloc_sb = cpool.tile([H, ROWS], _BF16)
            nc.scalar.dma_start(zTloc_sb[:], zTloc_d[:])
            w1aT_sb = cpool.tile([H, H], _BF16)
            nc.scalar.dma_start(w1aT_sb[:], w1aT_d[:])
            zT_sb = cpool.tile([H, N], _BF16)
            nc.sync.dma_start(zT_sb[:], zT_d[:])
            w1bT_sb = cpool.tile([H, H], _BF16)
            nc.sync.dma_start(w1bT_sb[:], w1bT_d[:])

            # Abias[k, a] = W1a @ z_loc.T + b1  (bias added below, after b1col)
            at_ps = psA.tile([H, ROWS], _F32, tag="at")
            nc.tensor.matmul(at_ps[:], w1aT_sb[:], zTloc_sb[:], start=True, stop=True)

            # Bt[k, j] = W1b @ z.T, cast f32 -> bf16 into SBUF.
            # Split 512+256 so each matmul output stays inside one 2KB PSUM
            # bank (the tile is bank-aligned; a 384-split would cross it).
            bt_ps = psA.tile([H, N], _F32, tag="btps")
            bt_sb = cpool.tile([H, N], _BF16)
            for lo, hi in ((0, 512), (512, N)):
                nc.tensor.matmul(
                    bt_ps[:, lo:hi],
                    w1bT_sb[:],
                    zT_sb[:, lo:hi],
                    start=True,
                    stop=True,
                )
                nc.vector.tensor_copy(bt_sb[:, lo:hi], bt_ps[:, lo:hi])

            b1col_sb = cpool.tile([H, 1], _F32)
            nc.scalar.dma_start(b1col_sb[:], b1col_d[:])
            b2row_sb = cpool.tile([1, ROWS], _BF16)
            nc.scalar.dma_start(b2row_sb[:], b2row_d[:])
            S_sb = cpool.tile([H, GRND, 32], _BF16)
            nc.sync.dma_start(S_sb[:], S_d[:])

            abias_sb = cpool.tile([H, ROWS], _F32)
            nc.vector.tensor_scalar_add(abias_sb[:], at_ps[:], b1col_sb[:])

            out_view = out_d.ap().rearrange("(r three) n -> three r n", three=NGRP)

            for t in range(GENS):
                ps = [
                    pspool.tile([ROWS, HALF], _F32, tag=f"ps{h}", name=f"ps{h}_{t}")
                    for h in range(2)
                ]
                # fold b2 in: ps[h] = b2  (start=True clears the bank)
                for h in range(2):
                    nc.tensor.matmul(
                        ps[h][:], b2row_sb[:], ones_sb[:], start=True, stop=False
                    )
                for q in range(GRND):
                    r = t * GRND + q
                    hts = [None] * NGRP
                    # engine split, measured on HW:
                    #   DVE tensor_scalar (2x_1P; the per-partition bias AP
                    #   blocks 4x mode)     ~413ns/tile
                    #   ACT activation(Relu, bias)            ~924ns/tile
                    #   GpSimd tensor_scalar: 11.3us/tile AND its shared-port
                    #   lock starves DVE -- unusable.
                    #   ACT reading Bt from PSUM stalls PE's PSUM writes
                    #   (MM 322->500ns), so ACT reads the SBUF copy too.
                    # -> 66:30 DVE:ACT split; pushing DVE harder (71:25)
                    # measured slower (dense DVE chains expose the SBUF
                    # read-write bubble, ~500ns/op).
                    engs = ("dve", "dve", "dve") if r in (10, 21) else ("dve", "dve", "act")
                    # issue the slow producers first
                    order = sorted(range(NGRP), key=lambda g: engs[g] == "dve")
                    for g in order:
                        a = NGRP * r + g
                        ht = hpool.tile([H, N], _BF16, tag="H", name=f"h{a}")
                        if engs[g] == "dve":
                            nc.vector.tensor_scalar(
                                ht[:], bt_sb[:], abias_sb[:, a : a + 1], 0.0,
                                add_op, max_op,
                            )
                        else:
                            nc.scalar.activation(
                                ht[:], bt_sb[:], relu,
                                bias=abias_sb[:, a : a + 1], scale=1.0,
                            )
                        hts[g] = ht
                    last = q == GRND - 1
                    # g-outer: the two half-matmuls of a group are adjacent,
                    # sharing one stationary load in the same column strip;
                    # cross-group concurrency still comes from the PE reorder
                    # window (different col groups overlap regardless)
                    for g in range(NGRP):
                        for h in range(2):
                            nc.tensor.matmul(
                                ps[h][32 * g : 32 * g + 32, :],
                                S_sb[:, q, :],
                                hts[g][:, h * HALF : (h + 1) * HALF],
                                start=False,
                                stop=last,
                            )
                # evict generation t: rows p = 32g + q -> DRAM row a = 3(16t+q)+g
                # gen0 on ACT (mid-loop, ACT has slack); gen1 split across
                # ACT+DVE so the two tail copies run in parallel
                ot = opool.tile([ROWS, N], _F32, tag="ot", name=f"ot{t}")
                for h in range(2):
                    if t == 0 or h == 0:
                        nc.scalar.activation(
                            ot[:, h * HALF : (h + 1) * HALF], ps[h][:], copyf
                        )
                    else:
                        nc.vector.tensor_copy(
                            ot[:, h * HALF : (h + 1) * HALF], ps[h][:]
                        )
                for g in range(NGRP):
                    eng = nc.sync if g != 1 else nc.scalar
                    eng.dma_start(
                        out_view[g, t * GRND : (t + 1) * GRND],
                        ot[32 * g : 32 * g + GRND, :],
                    )

    nc.compile()
    return nc


def _get_nc():
    if "nc" not in _cache:
        _cache["nc"] = _build()
    return _cache["nc"]


def _prep_in_maps(z, W1, b1, W2, b2):
    z = np.asarray(z, np.float32)
    W1 = np.asarray(W1, np.float32)
    b1 = np.asarray(b1, np.float32)
    W2 = np.asarray(W2, np.float32)
    b2 = np.asarray(b2, np.float32)

    bf = ml_dtypes.bfloat16
    zT = np.ascontiguousarray(z.T)                          # [H, N]
    w1aT = np.ascontiguousarray(W1[:, :H].T).astype(bf)     # [c, k]
    w1bT = np.ascontiguousarray(W1[:, H:].T).astype(bf)     # [c, k]
    b1col = np.ascontiguousarray(b1.reshape(H, 1))
    b2row = np.full((1, ROWS), float(b2[0]), np.float32).astype(bf)

    # zero-padded stationary (shared by col groups and generations):
    # S[k, q, q] = W2[0, k]
    S = np.zeros((H, GRND, 32), np.float32)
    q = np.arange(GRND)
    S[:, q, q] = W2[0][:, None]
    S = S.astype(bf)

    zT_bf = zT.astype(bf)
    in_maps = []
    for c in range(NCORES):
        in_maps.append(
            {
                "zT": zT_bf,
                "zTloc": np.ascontiguousarray(
                    zT[:, c * ROWS : (c + 1) * ROWS]
                ).astype(bf),
                "w1aT": w1aT,
                "w1bT": w1bT,
                "b1col": b1col,
                "b2row": b2row,
                "S": S,
            }
        )
    return in_maps


def _assemble(results):
    full = np.concatenate(
        [np.asarray(results[c]["out"], np.float32) for c in range(NCORES)], axis=0
    )  # [N, N] scores incl. diagonal
    mask = ~np.eye(N, dtype=bool)
    return full[mask]  # pair-major order: i-major, j ascending, j != i


def run(z, W1, b1, W2, b2, trace=False, tmpdir=None):
    nc = _get_nc()
    in_maps = _prep_in_maps(z, W1, b1, W2, b2)
    res = run_bass_kernel_spmd(
        nc, in_maps, core_ids=list(range(NCORES)), trace=trace, tmpdir=tmpdir
    )
    return _assemble(res.results), res


def kernel(z, W1, b1, W2, b2):
    out, _ = run(z, W1, b1, W2, b2, trace=False)
    return out


# revision 34
# speedup vs baseline: 1.0087x; 1.0087x over previous
"""Trainium2 Bass kernel for ArcDecoder pair scoring.

Reference computation (N=768 nodes, H=128 features):
    pairs (i, j), i != j:  out[i,j] = W2 @ relu(W1a @ z_i + W1b @ z_j + b1) + b2
where W1 = [W1a | W1b] ([128, 256] split along input dim).

Strategy (pure data parallel over 8 NeuronCores, no collectives):
  - core c owns i-rows [96c, 96c+96); output rows are contiguous in the
    final pair-major ordering, so gather = concat + drop diagonal.
  - device layout puts the hidden dim k on SBUF partitions:
      Abias[k, a] = W1a @ z_loc.T + b1     (per-core [128, 96])
      Bt[k, j]    = W1b @ z.T              (replicated [128, 768], bf16)
    per a: H_a = relu(Bt + Abias[:, a])    (split 66:30 DVE / ACT)
    out[a, :] = W2 @ H_a (+ b2 via a K=1 matmul into the same PSUM
    accumulation).  TensorE uses a zero-padded W2 stationary and 3
    column groups running concurrently; the row for a = 3r + g lands on
    PSUM partition 32g + (r % 16).  Two PSUM generations (16 rounds
    each) so eviction/output-DMA of gen 0 overlaps gen 1 compute.
"""

import numpy as np
import ml_dtypes

import concourse.bass as bass
import concourse.tile as tile
from concourse import bacc, mybir
from concourse.bass_utils import run_bass_kernel_spmd

N = 768
H = 128
NCORES = 8
ROWS = N // NCORES          # 96 i-rows per core
NGRP = 3                    # PE column groups (PSUM partitions 32g..32g+31)
RND = ROWS // NGRP          # 32 rounds; round r, group g handles a = 3r + g
GENS = 2
GRND = RND // GENS          # 16 rounds per PSUM generation
HALF = N // 2               # 384, PSUM bank limit for f32 is 512

_F32 = mybir.dt.float32
_BF16 = mybir.dt.bfloat16

_cache = {}


def _build():
    nc = bacc.Bacc(
        "TRN2",
        target_bir_lowering=False,
        debug=False,
        enable_asserts=False,
        num_devices=NCORES,
    )

    zT_d = nc.dram_tensor("zT", [H, N], _BF16, kind="ExternalInput")
    zTloc_d = nc.dram_tensor("zTloc", [H, ROWS], _BF16, kind="ExternalInput")
    w1aT_d = nc.dram_tensor("w1aT", [H, H], _BF16, kind="ExternalInput")
    w1bT_d = nc.dram_tensor("w1bT", [H, H], _BF16, kind="ExternalInput")
    b1col_d = nc.dram_tensor("b1col", [H, 1], _F32, kind="ExternalInput")
    b2row_d = nc.dram_tensor("b2row", [1, ROWS], _BF16, kind="ExternalInput")
    S_d = nc.dram_tensor("S", [H, GRND, 32], _BF16, kind="ExternalInput")
    out_d = nc.dram_tensor("out", [ROWS, N], _F32, kind="ExternalOutput")

    relu = mybir.ActivationFunctionType.Relu
    copyf = mybir.ActivationFunctionType.Copy
    add_op = mybir.AluOpType.add
    max_op = mybir.AluOpType.max

    with tile.TileContext(nc) as tc:
        with (
            tc.tile_pool(name="const", bufs=1) as cpool,
            tc.tile_pool(name="hpool", bufs=8) as hpool,
            tc.tile_pool(name="opool", bufs=2) as opool,
            tc.tile_pool(name="psA", bufs=1, space=bass.MemorySpace.PSUM) as psA,
            tc.tile_pool(name="psum", bufs=2, space=bass.MemorySpace.PSUM) as pspool,
        ):
            # ACT spline-table prewarm (Relu + Copy) so any one-time
            # ACT_TABLE_LOAD overlaps the input DMAs.
            scratch = cpool.tile([1, 8], _F32, tag="scratch")
            nc.gpsimd.memset(scratch[:], 0.0)
            nc.scalar.activation(scratch[:], scratch[:], relu)
            nc.scalar.activation(scratch[:], scratch[:], copyf)

            # all-ones moving row for the b2 fold-in matmul
            ones_sb = cpool.tile([1, HALF], _BF16)
            nc.gpsimd.memset(ones_sb[:], 1.0)

            # inputs, spread across the two HWDGE queues (sync, scalar).
            # Tile's per-queue clock makes a consumer wait for every DMA
            # emitted before it in program order, so emit the setup matmuls
            # immediately after the DMAs they actually need and push the
            # small late-use DMAs (b1col/b2row/S) after them.
            zTloc_sb = cpool.tile([H, ROWS], _BF16)
            nc.scalar.dma_start(zTloc_sb[:], zTloc_d[:])
            w1aT_sb = cpool.tile([H, H], _BF16)
            nc.scalar.dma_start(w1aT_sb[:], w1aT_d[:])
            zT_sb = cpool.tile([H, N], _BF16)
            nc.sync.dma_start(zT_sb[:], zT_d[:])
            w1bT_sb = cpool.tile([H, H], _BF16)
            nc.sync.dma_start(w1bT_sb[:], w1bT_d[:])

            # Abias[k, a] = W1a @ z_loc.T + b1  (bias added below, after b1col)
            at_ps = psA.tile([H, ROWS], _F32, tag="at")
            nc.tensor.matmul(at_ps[:], w1aT_sb[:], zTloc_sb[:], start=True, stop=True)

            # Bt[k, j] = W1b @ z.T, cast f32 -> bf16 into SBUF.
            # Split 512+256 so each matmul output stays inside one 2KB PSUM
            # bank (the tile is bank-aligned; a 384-split would cross it).
            bt_ps = psA.tile([H, N], _F32, tag="btps")
            bt_sb = cpool.tile([H, N], _BF16)
            for lo, hi in ((0, 512), (512, N)):
                nc.tensor.matmul(
                    bt_ps[:, lo:hi],
                    w1bT_sb[:],
                    zT_sb[:, lo:hi],
                    start=True,
                    stop=True,
                )
                nc.vector.tensor_copy(bt_sb[:, lo:hi], bt_ps[:, lo:hi])

            b1col_sb = cpool.tile([H, 1], _F32)
            nc.scalar.dma_start(b1col_sb[:], b1col_d[:])
            b2row_sb = cpool.tile([1, ROWS], _BF16)
            nc.scalar.dma_start(b2row_sb[:], b2row_d[:])
            S_sb = cpool.tile([H, GRND, 32], _BF16)
            nc.sync.dma_start(S_sb[:], S_d[:])

            abias_sb = cpool.tile([H, ROWS], _F32)
            nc.vector.tensor_scalar_add(abias_sb[:], at_ps[:], b1col_sb[:])

            out_view = out_d.ap().rearrange("(r three) n -> three r n", three=NGRP)

            for t in range(GENS):
                ps = [
                    pspool.tile([ROWS, HALF], _F32, tag=f"ps{h}", name=f"ps{h}_{t}")
                    for h in range(2)
                ]
                # fold b2 in: ps[h] = b2  (start=True clears the bank)
                for h in range(2):
                    nc.tensor.matmul(
                        ps[h][:], b2row_sb[:], ones_sb[:], start=True, stop=False
                    )
                for q in range(GRND):
                    r = t * GRND + q
                    hts = [None] * NGRP
                    # engine split, measured on HW:
                    #   DVE tensor_scalar (2x_1P; the per-partition bias AP
                    #   blocks 4x mode)     ~413ns/tile
                    #   ACT activation(Relu, bias)            ~924ns/tile
                    #   GpSimd tensor_scalar: 11.3us/tile AND its shared-port
                    #   lock starves DVE -- unusable.
                    #   ACT reading Bt from PSUM stalls PE's PSUM writes
                    #   (MM 322->500ns), so ACT reads the SBUF copy too.
                    # -> 66:30 DVE:ACT split; pushing DVE harder (71:25)
                    # measured slower (dense DVE chains expose the SBUF
                    # read-write bubble, ~500ns/op).
                    engs = ("dve", "dve", "dve") if r in (10, 21) else ("dve", "dve", "act")
                    # issue the slow producers first
                    order = sorted(range(NGRP), key=lambda g: engs[g] == "dve")
                    for g in order:
                        a = NGRP * r + g
                        ht = hpool.tile([H, N], _BF16, tag="H", name=f"h{a}")
                        if engs[g] == "dve":
                            nc.vector.tensor_scalar(
                                ht[:], bt_sb[:], abias_sb[:, a : a + 1], 0.0,
                                add_op, max_op,
                            )
                        else:
                            nc.scalar.activation(
                                ht[:], bt_sb[:], relu,
                                bias=abias_sb[:, a : a + 1], scale=1.0,
                            )
                        hts[g] = ht
                    last = q == GRND - 1
                    for h in range(2):
                        for g in range(NGRP):
                            nc.tensor.matmul(
                                ps[h][32 * g : 32 * g + 32, :],
                                S_sb[:, q, :],
                                hts[g][:, h * HALF : (h + 1) * HALF],
                                start=False,
                                stop=last,
                            )
                # evict generation t: rows p = 32g + q -> DRAM row a = 3(16t+q)+g
                # gen0 on ACT (mid-loop, ACT has slack); gen1 split across
                # ACT+DVE so the two tail copies run in parallel
                ot = opool.tile([ROWS, N], _F32, tag="ot", name=f"ot{t}")
                for h in range(2):
                    if t == 0 or h == 0:
                        nc.scalar.activation(
                            ot[:, h * HALF : (h + 1) * HALF], ps[h][:], copyf
                        )
                    else:
                        nc.vector.tensor_copy(
                            ot[:, h * HALF : (h + 1) * HALF], ps[h][:]
                        )
                for g in range(NGRP):
                    eng = nc.sync if g != 1 else nc.scalar
                    eng.dma_start(
                        out_view[g, t * GRND : (t + 1) * GRND],
                        ot[32 * g : 32 * g + GRND, :],
                    )

    nc.compile()
    return nc


def _get_nc():
    if "nc" not in _cache:
        _cache["nc"] = _build()
    return _cache["nc"]


def _prep_in_maps(z, W1, b1, W2, b2):
    z = np.asarray(z, np.float32)
    W1 = np.asarray(W1, np.float32)
    b1 = np.asarray(b1, np.float32)
    W2 = np.asarray(W2, np.float32)
    b2 = np.asarray(b2, np.float32)

    bf = ml_dtypes.bfloat16
    zT = np.ascontiguousarray(z.T)                          # [H, N]
    w1aT = np.ascontiguousarray(W1[:, :H].T).astype(bf)     # [c, k]
    w1bT = np.ascontiguousarray(W1[:, H:].T).astype(bf)     # [c, k]
    b1col = np.ascontiguousarray(b1.reshape(H, 1))
    b2row = np.full((1, ROWS), float(b2[0]), np.float32).astype(bf)

    # zero-padded stationary (shared by col groups and generations):
    # S[k, q, q] = W2[0, k]
    S = np.zeros((H, GRND, 32), np.float32)
    q = np.arange(GRND)
    S[:, q, q] = W2[0][:, None]
    S = S.astype(bf)

    zT_bf = zT.astype(bf)
    in_maps = []
    for c in range(NCORES):
        in_maps.append(
            {
                "zT": zT_bf,
                "zTloc": np.ascontiguousarray(
                    zT[:, c * ROWS : (c + 1) * ROWS]
                ).astype(bf),
                "w1aT": w1aT,
                "w1bT": w1bT,
                "b1col": b1col,
                "b2row": b2row,
                "S": S,
            }
        )
    return in_maps


def _assemble(results):
    full = np.concatenate(
        [np.asarray(results[c]["out"], np.float32) for c in range(NCORES)], axis=0
    )  # [N, N] scores incl. diagonal
    mask = ~np.eye(N, dtype=bool)
    return full[mask]  # pair-major order: i-major, j ascending, j != i


def run(z, W1, b1, W2, b2, trace=False, tmpdir=None):
    nc = _get_nc()
    in_maps = _prep_in_maps(z, W1, b1, W2, b2)
    res = run_bass_kernel_spmd(
        nc, in_maps, core_ids=list(range(NCORES)), trace=trace, tmpdir=tmpdir
    )
    return _assemble(res.results), res


def kernel(z, W1, b1, W2, b2):
    out, _ = run(z, W1, b1, W2, b2, trace=False)
    return out
